# revision 6
# baseline (speedup 1.0000x reference)
"""DRM attention kernel for 8 Trainium2 NeuronCores — v14.

v2 -> v3: pair-tile PSUM layout (2-bank tiles) so ACT squares cover two
uk banks per instruction; all-pairs metric stage hoisted before scores;
software-pipelined score loop (spans -> squares -> tree -> exp -> attnV
one block behind) to keep the PE stream dense for HAM; copies spread
across DVE/ACT/GPS by phase.
"""

import numpy as np
import ml_dtypes

B, T, DM = 2, 512, 1024
H, DH = 16, 64
D, R = 32, 4
TEMP_MIN = 0.5
NCORE = 8
TC = 4

_CACHE = {}
BF16 = ml_dtypes.bfloat16


def _build(temp):
    import concourse.bass as bass
    import concourse.tile as tile
    from concourse import mybir, bacc

    f32 = mybir.dt.float32
    f32r = mybir.dt.float32r
    bf16 = mybir.dt.bfloat16
    PSUM = bass.MemorySpace.PSUM
    Act = mybir.ActivationFunctionType

    it = -1.0 / max(temp, TEMP_MIN)

    nc = bacc.Bacc("TRN2", target_bir_lowering=False, debug=False)

    xt_d = nc.dram_tensor("xt", [DM, T], bf16, kind="ExternalInput")
    wqk_d = nc.dram_tensor("wqk", [DM, 512], bf16, kind="ExternalInput")
    wv_d = nc.dram_tensor("wv", [DM, 256], bf16, kind="ExternalInput")
    wo_d = nc.dram_tensor("wo", [256, DM], bf16, kind="ExternalInput")
    bqkA_d = nc.dram_tensor("bqkA", [128, 128], bf16, kind="ExternalInput")
    bqkAn_d = nc.dram_tensor("bqkAn", [128, 128], bf16, kind="ExternalInput")
    bqkB_d = nc.dram_tensor("bqkB", [128, 128], bf16, kind="ExternalInput")
    cosr_d = nc.dram_tensor("cosr", [128, T], bf16, kind="ExternalInput")
    sinr_d = nc.dram_tensor("sinr", [128, T], bf16, kind="ExternalInput")
    wm4_d = nc.dram_tensor("wm4", [128, 128], f32r, kind="ExternalInput")
    i4q4_d = nc.dram_tensor("i4q4", [128, 128], f32r, kind="ExternalInput")
    i4kn2_d = nc.dram_tensor("i4kn2", [128, 128], f32r, kind="ExternalInput")
    gsum_d = nc.dram_tensor("gsum", [128, 128], f32r, kind="ExternalInput")
    bsum4_d = nc.dram_tensor("bsum4", [128, 128], f32r, kind="ExternalInput")
    maskd_d = nc.dram_tensor("maskd", [128, 128], bf16, kind="ExternalInput")
    y_d = nc.dram_tensor("y", [T, DM], bf16, kind="ExternalOutput")

    with tile.TileContext(nc) as tc:
        with (
            tc.tile_pool(name="const", bufs=1) as cpool,
            tc.tile_pool(name="sig", bufs=1) as sigpool,
            tc.tile_pool(name="m", bufs=8) as mpool,
            tc.tile_pool(name="met", bufs=2) as metpool,
            tc.tile_pool(name="sc", bufs=2) as scpool,
            tc.tile_pool(name="f", bufs=4) as fpool,
            tc.tile_pool(name="pt", bufs=4) as ptpool,
            tc.tile_pool(name="out", bufs=2) as opool,
            tc.tile_pool(name="ps", bufs=1, space=PSUM) as psp,
        ):
            # ---- constants / weights ----
            xt = [cpool.tile([128, T], bf16, tag=f"xt{k}", name=f"xt{k}")
                  for k in range(8)]
            wqk = [cpool.tile([128, 512], bf16, tag=f"wqk{k}", name=f"wqk{k}")
                   for k in range(8)]
            wv = [cpool.tile([128, 256], bf16, tag=f"wv{k}", name=f"wv{k}")
                  for k in range(8)]
            wo = [cpool.tile([128, DM], bf16, tag=f"wo{p}", name=f"wo{p}")
                  for p in range(2)]
            bqkA = cpool.tile([128, 128], bf16, tag="bqkA")
            bqkAn = cpool.tile([128, 128], bf16, tag="bqkAn")
            bqkB = cpool.tile([128, 128], bf16, tag="bqkB")
            cosr = cpool.tile([128, T], bf16, tag="cosr")
            sinr = cpool.tile([128, T], bf16, tag="sinr")
            wm4 = cpool.tile([128, 128], f32r, tag="wm4")
            i4q4 = cpool.tile([128, 128], f32r, tag="i4q4")
            i4kn2 = cpool.tile([128, 128], f32r, tag="i4kn2")
            gsum = cpool.tile([128, 128], f32r, tag="gsum")
            bsum4 = cpool.tile([128, 128], f32r, tag="bsum4")
            maskd = cpool.tile([128, 128], bf16, tag="maskd")
            ones64 = cpool.tile([1, 64], f32r, tag="ones64")
            dmy = cpool.tile([128, 128], bf16, tag="dmy")
            onesv = cpool.tile([64, 2], f32r, tag="onesv")
            vext = cpool.tile([128, TC, 260], bf16, tag="vext")
            stacked = [cpool.tile([128, T], bf16, tag=f"stk{p}", name=f"stk{p}")
                       for p in range(2)]

            xt_r = xt_d.ap().rearrange("(k p) t -> k p t", p=128)
            wqk_r = wqk_d.ap().rearrange("(k p) m -> k p m", p=128)
            wv_r = wv_d.ap().rearrange("(k p) m -> k p m", p=128)
            wo_r = wo_d.ap().rearrange("(k p) m -> k p m", p=128)
            for k in range(8):
                nc.sync.dma_start(xt[k][:], xt_r[k])
                nc.sync.dma_start(wqk[k][:], wqk_r[k])
            nc.sync.dma_start(cosr[:], cosr_d.ap())
            nc.sync.dma_start(sinr[:], sinr_d.ap())
            nc.sync.dma_start(bqkA[:], bqkA_d.ap())
            nc.sync.dma_start(bqkAn[:], bqkAn_d.ap())
            nc.sync.dma_start(bqkB[:], bqkB_d.ap())
            for k in range(8):
                nc.sync.dma_start(wv[k][:], wv_r[k])
            nc.sync.dma_start(wm4[:], wm4_d.ap())
            nc.sync.dma_start(i4q4[:], i4q4_d.ap())
            nc.sync.dma_start(i4kn2[:], i4kn2_d.ap())
            nc.sync.dma_start(gsum[:], gsum_d.ap())
            nc.sync.dma_start(bsum4[:], bsum4_d.ap())
            nc.sync.dma_start(maskd[:], maskd_d.ap())
            for p in range(2):
                nc.sync.dma_start(wo[p][:], wo_r[p])
            nc.gpsimd.memset(ones64[:].bitcast(f32), 1.0)
            nc.gpsimd.memset(dmy[:], 0.0)
            nc.gpsimd.memset(onesv[:].bitcast(f32), 1.0)
            nc.gpsimd.memset(vext[:], 1.0)

            def pU(name):
                return psp.tile([128, 2, 512], f32, tag="pU", bufs=2, name=name)

            def pS(shape, name):
                return psp.tile(shape, f32, tag="pS", bufs=2, name=name)

            def pO(shape, name):
                return psp.tile(shape, f32, tag="pO", bufs=2, name=name)

            # ---- QK projection: per pair one [128, 2, 512] pair tile ----
            ropes = {}
            for p in range(2):
                qk = pU(f"qk{p}")
                for s in range(2):
                    m = 2 * p + s
                    for k in range(8):
                        nc.tensor.matmul(
                            qk[:, s, :], wqk[k][:, m * 128:(m + 1) * 128],
                            xt[k][:], start=(k == 0), stop=(k == 7),
                            skip_group_check=True)
                m1 = mpool.tile([128, T], bf16, tag="m", name=f"m1_{p}")
                m2 = mpool.tile([128, T], bf16, tag="m", name=f"m2_{p}")
                m3 = mpool.tile([128, T], bf16, tag="m", name=f"m3_{p}")
                m4 = mpool.tile([128, T], bf16, tag="m", name=f"m4_{p}")
                nc.vector.tensor_mul(m1[:], qk[:, 0, :], cosr[:])
                nc.vector.tensor_mul(m2[:], qk[:, 1, :], sinr[:])
                nc.vector.tensor_mul(m3[:], qk[:, 0, :], sinr[:])
                nc.vector.tensor_mul(m4[:], qk[:, 1, :], cosr[:])
                ropes[p] = (m1, m2, m3, m4)

            # ---- V projection ----
            for jc in range(TC):
                v_ps = pO([128, 256], f"v{jc}")
                for k in range(8):
                    nc.tensor.matmul(
                        v_ps[:], xt[k][:, jc * 128:(jc + 1) * 128], wv[k][:],
                        start=(k == 0), stop=(k == 7))
                vsl = vext[:, jc, :].rearrange("p (h c) -> p h c", c=65)
                nc.vector.tensor_copy(
                    vsl[:, :, 0:64],
                    v_ps[:].rearrange("p (h c) -> p h c", c=64))

            # ---- qm/km + sigmoid ----
            sig = []
            for p in range(2):
                m1, m2, m3, m4 = ropes[p]
                qkm_ps = pS([128, T], f"qkm{p}")
                nc.tensor.matmul(qkm_ps[:], bqkA[:], m1[:], start=True, stop=False)
                nc.tensor.matmul(qkm_ps[:], bqkAn[:], m2[:], start=False, stop=False)
                nc.tensor.matmul(qkm_ps[:], bqkB[:], m3[:], start=False, stop=False)
                nc.tensor.matmul(qkm_ps[:], bqkB[:], m4[:], start=False, stop=True)
                sg = sigpool.tile([128, T], f32r, tag=f"sig{p}", name=f"sig{p}")
                nc.scalar.activation(sg[:], qkm_ps[:], Act.Sigmoid)
                sig.append(sg)

            # ---- metric stage, both pairs up front ----
            # uu = -1/2 U [(r,d), i]; km = -2 km replicated; qm replicated;
            # gt = qm + w'; ksq bias.
            pairdat = []
            for p in range(2):
                sg = sig[p]
                up = pU(f"uu{p}")
                for e in range(2):
                    nc.tensor.matmul(up[:, e, :], wm4[32 * e:32 * e + 32, :],
                                     sg[32 * e:32 * e + 32, :],
                                     start=True, stop=True,
                                     skip_group_check=True)
                uu = metpool.tile([128, 2, T], f32r, tag="uu", bufs=2,
                                  name=f"uu{p}")
                nc.vector.tensor_copy(uu[:], up[:])
                kp = pU(f"km{p}")
                for e in range(2):
                    nc.tensor.matmul(kp[:, e, :],
                                     i4kn2[64 + 32 * e:96 + 32 * e, :],
                                     sg[64 + 32 * e:96 + 32 * e, :],
                                     start=True, stop=True,
                                     tile_position=(64 + 32 * e, 0),
                                     skip_group_check=True)
                km = metpool.tile([128, 2, T], f32r, tag="km", bufs=2,
                                  name=f"km{p}")
                nc.vector.tensor_copy(km[:], kp[:])
                kmsq2 = metpool.tile([64, T], f32r, tag="kmsq", bufs=2,
                                     name=f"kmsq{p}")
                nc.scalar.activation(kmsq2[:], sg[64:128, :], Act.Square)
                biases = []
                for e in range(2):
                    ksq_ps = pS([128, 2 * TC], f"ksq{p}{e}")
                    for jc in range(TC):
                        nc.tensor.matmul(
                            ksq_ps[:, 2 * jc:2 * jc + 2],
                            kmsq2[32 * e:32 * e + 32,
                                  jc * 128:(jc + 1) * 128],
                            onesv[32 * e:32 * e + 32, 0:2],
                            start=True, stop=True, skip_group_check=True)
                    bias_h = metpool.tile([128, 2 * TC], f32, tag="bias",
                                          bufs=4, name=f"bias{p}{e}")
                    nc.scalar.mul(bias_h[:], ksq_ps[:], it)
                    biases.append(bias_h)
                gts = []
                for e in range(2):
                    qp = pS([128, T], f"qm{p}{e}")
                    nc.tensor.matmul(qp[:], i4q4[32 * e:32 * e + 32, :],
                                     sg[32 * e:32 * e + 32, :],
                                     start=True, stop=True)
                    qm = metpool.tile([128, T], f32r, tag="qm", bufs=4,
                                      name=f"qm{p}{e}")
                    nc.scalar.copy(qm[:], qp[:])
                    tmpc = metpool.tile([128, T], f32r, tag="tmpc", bufs=2,
                                        name=f"tmpc{p}{e}")
                    nc.gpsimd.tensor_mul(tmpc[:], uu[:, e, :], qm[:])
                    uq_ps = pS([128, T], f"uq{p}{e}")
                    nc.tensor.matmul(uq_ps[:], gsum[:], tmpc[:],
                                     start=True, stop=True)
                    tmp2c = metpool.tile([128, T], f32r, tag="tmp2c", bufs=2,
                                         name=f"tmp2c{p}{e}")
                    nc.vector.tensor_mul(tmp2c[:], uu[:, e, :], uq_ps[:])
                    wp_ps = pS([128, T], f"wp{p}{e}")
                    nc.tensor.matmul(wp_ps[:], bsum4[:], tmp2c[:],
                                     start=True, stop=True)
                    gt = metpool.tile([128, T], f32r, tag="gt", bufs=4,
                                      name=f"gt{p}{e}")
                    nc.vector.tensor_add(gt[:], qm[:], wp_ps[:])
                    gts.append(gt)
                    for _ in range(4):
                        nc.tensor.ldweights(weights=dmy[:])
                pairdat.append((uu, km, gts, biases))

            # ---- scores: exp and attnV software-pipelined one block
            # behind the spans/squares/tree so the ACT FIFO never stalls
            # head-of-line on a tree dependency ----
            for p in range(2):
                uu, km, gts, biases = pairdat[p]
                ot = {}
                for e in range(2):
                    ot[e] = pO([128, T], f"ot{p}{e}", )
                expq = []   # blocks awaiting exp
                attq = []   # blocks awaiting attnV
                def flush_att():
                    while attq:
                        (ae, apt, aioff, ani) = attq.pop(0)
                        hl = 2 * p + ae
                        nc.tensor.matmul(
                            ot[ae][:65, aioff:],
                            vext[:, aioff // 128, hl * 65:(hl + 1) * 65],
                            apt[:, :ani],
                            start=(aioff == 0), stop=(aioff == 384),
                            skip_group_check=True)
                def flush_exp():
                    (xe, xsl, xioff, xni, xjc) = expq.pop(0)
                    pt = ptpool.tile([128, T], bf16, tag="pt",
                                     name=f"pt{p}{xe}{xjc}")
                    nc.scalar.activation(pt[:, :xni], xsl[:, :xni],
                                         Act.Exp, scale=it,
                                         bias=biases[xe][:, 2 * xjc:2 * xjc + 1])
                    nc.gpsimd.tensor_mul(pt[:, 0:128], pt[:, 0:128],
                                         maskd[:])
                    attq.append((xe, pt, xioff, xni))
                for jc in range(TC):
                    ioff = 128 * jc
                    ni = T - ioff
                    for e in range(2):
                        ukA = pU(f"ukA{p}{e}{jc}")
                        for r in range(2):
                            nc.tensor.matmul(
                                ukA[:, r, :ni],
                                km[32 * r:32 * r + 32, e,
                                   ioff:ioff + 128],
                                uu[32 * r:32 * r + 32, e, ioff:],
                                start=True, stop=True,
                                skip_group_check=True)
                        sl = pS([128, T], f"sl{p}{e}{jc}")
                        nc.tensor.matmul(
                            sl[:, :ni],
                            km[64:96, e, ioff:ioff + 128],
                            gts[e][64:96, ioff:],
                            start=True, stop=True)
                        ukB = pU(f"ukB{p}{e}{jc}")
                        for r in range(2):
                            nc.tensor.matmul(
                                ukB[:, r, :ni],
                                km[64 + 32 * r:96 + 32 * r, e,
                                   ioff:ioff + 128],
                                uu[64 + 32 * r:96 + 32 * r, e, ioff:],
                                start=True, stop=True,
                                tile_position=(64 + 32 * r, 0),
                                skip_group_check=True)
                        # attnV from two blocks ago fills the PE here
                        flush_att()
                        sqA = fpool.tile([128, 2, T], bf16, tag="f",
                                         name=f"sqA{p}{e}{jc}")
                        sqB = fpool.tile([128, 2, T], bf16, tag="f",
                                         name=f"sqB{p}{e}{jc}")
                        nc.scalar.square(sqA[:, :, :ni], ukA[:, :, :ni])
                        nc.scalar.square(sqB[:, :, :ni], ukB[:, :, :ni])
                        psum = ptpool.tile([128, 2, T], bf16, tag="psum",
                                           name=f"ps{p}{e}{jc}")
                        nc.vector.tensor_add(psum[:, :, :ni], sqA[:, :, :ni],
                                             sqB[:, :, :ni])
                        tt = ptpool.tile([128, T], bf16, tag="tt",
                                         name=f"tt{p}{e}{jc}")
                        nc.gpsimd.tensor_add(tt[:, :ni], psum[:, 0, :ni],
                                             psum[:, 1, :ni])
                        nc.vector.tensor_add(sl[:, :ni], tt[:, :ni],
                                             sl[:, :ni])
                        for _ in range(6):
                            nc.tensor.ldweights(weights=dmy[:])
                        expq.append((e, sl, ioff, ni, jc))
                        # exp for the PREVIOUS block (ACT stays fed with
                        # this block's squares while its tree completes)
                        if len(expq) > 1:
                            flush_exp()
                while expq:
                    flush_exp()
                flush_att()

                # normalize
                for e in range(2):
                    den = scpool.tile([1, T], f32r, tag="den", name=f"dn{p}{e}")
                    nc.vector.tensor_copy(den[:], ot[e][64:65, :])
                    bc_ps = pS([64, T], f"bc{p}{e}")
                    nc.tensor.matmul(bc_ps[:], ones64[:], den[:],
                                     start=True, stop=True)
                    bc = scpool.tile([64, T], f32, tag="bc", name=f"bc{p}{e}")
                    nc.vector.reciprocal_approx_fast(out=bc[:], in_=bc_ps[:])
                    nc.vector.tensor_mul(stacked[p][64 * e:64 * e + 64, :],
                                         ot[e][:64, :], bc[:])

            # ---- output projection ----
            for ic in range(TC):
                for ncn in range(2):
                    y_ps = pS([128, 512], f"y{ic}{ncn}")
                    nc.tensor.matmul(
                        y_ps[:], stacked[0][:, ic * 128:(ic + 1) * 128],
                        wo[0][:, ncn * 512:(ncn + 1) * 512],
                        start=True, stop=False)
                    nc.tensor.matmul(
                        y_ps[:], stacked[1][:, ic * 128:(ic + 1) * 128],
                        wo[1][:, ncn * 512:(ncn + 1) * 512],
                        start=False, stop=True)
                    y_sb = opool.tile([128, 512], bf16, tag="ysb",
                                      name=f"ysb{ic}{ncn}")
                    if ncn == 0:
                        nc.scalar.copy(y_sb[:], y_ps[:])
                    else:
                        nc.vector.tensor_copy(y_sb[:], y_ps[:])
                    nc.sync.dma_start(
                        y_d.ap()[ic * 128:(ic + 1) * 128,
                                 ncn * 512:(ncn + 1) * 512],
                        y_sb[:])

    nc.compile()
    return nc


def _r32(a):
    u = np.ascontiguousarray(a, np.float32).view(np.uint32).astype(np.uint64)
    u = (u + 0x7FF + ((u >> 12) & 1)) & 0xFFFFF000
    return u.astype(np.uint32).view(np.float32)


def _bf(a):
    return np.ascontiguousarray(np.asarray(a, np.float32)).astype(BF16)


def _rope_tables():
    inv_freq = 1.0 / (10000.0 ** (np.arange(0, DH, 2, dtype=np.float32) / DH))
    t = np.arange(T, dtype=np.float32)
    freqs = t[:, None] * inv_freq[None, :]
    return np.cos(freqs), np.sin(freqs)


def _prep_inputs(x, Wq, Wk, Wv, Wo, Wqm, Wkm, Wmetric, temperature):
    x = np.asarray(x, np.float32)
    Wq, Wk, Wv, Wo = (np.asarray(w, np.float32) for w in (Wq, Wk, Wv, Wo))
    Wqm, Wkm = np.asarray(Wqm, np.float32), np.asarray(Wkm, np.float32)
    Wmetric = np.asarray(Wmetric, np.float32)

    cosf, sinf = _rope_tables()
    cosr = _bf(np.tile(cosf.T, (4, 1)))
    sinr = _bf(np.tile(sinf.T, (4, 1)))

    bqkA = np.zeros((128, 128), np.float32)
    bqkB = np.zeros((128, 128), np.float32)
    for ee in range(2):
        bqkA[64 * ee:64 * ee + 32, 32 * ee:32 * ee + 32] = Wqm[0:32]
        bqkA[64 * ee + 32:64 * ee + 64, 64 + 32 * ee:96 + 32 * ee] = Wkm[0:32]
        bqkB[64 * ee:64 * ee + 32, 32 * ee:32 * ee + 32] = Wqm[32:64]
        bqkB[64 * ee + 32:64 * ee + 64, 64 + 32 * ee:96 + 32 * ee] = Wkm[32:64]

    wm = -0.5 * np.ascontiguousarray(
        Wmetric.reshape(D, D, R).transpose(0, 2, 1).reshape(D, D * R))
    wm4 = _r32(np.tile(wm, (4, 1)))

    i4 = np.tile(np.eye(D, dtype=np.float32), (1, 4))
    i4q4 = _r32(np.tile(i4, (4, 1)))
    i4kn2 = _r32(np.tile(-2.0 * i4, (4, 1)))

    gsum = np.zeros((128, 128), np.float32)
    for a in range(128):
        for bcol in range(128):
            if a // 32 == bcol // 32:
                gsum[a, bcol] = 1.0
    bsum4 = np.zeros((128, 128), np.float32)
    for a in range(128):
        for m in range(128):
            if a % 32 == m % 32:
                bsum4[a, m] = 4.0

    jj, ii = np.meshgrid(np.arange(128), np.arange(128), indexing="ij")
    maskd = _bf((jj <= ii).astype(np.float32))

    in_maps = []
    for c in range(NCORE):
        b, g = c // 4, c % 4
        lh0 = 4 * g
        wqk = np.empty((DM, 512), np.float32)
        for p in range(2):
            for s in range(2):
                m = 2 * p + s
                for ee in range(2):
                    h = lh0 + 2 * p + ee
                    cq = Wq[:, h * 64 + 32 * s: h * 64 + 32 * s + 32]
                    ck = Wk[:, h * 64 + 32 * s: h * 64 + 32 * s + 32]
                    wqk[:, m * 128 + 64 * ee: m * 128 + 64 * ee + 32] = cq
                    wqk[:, m * 128 + 64 * ee + 32: m * 128 + 64 * ee + 64] = ck
        in_maps.append({
            "xt": _bf(x[b].T),
            "wqk": _bf(wqk),
            "wv": _bf(Wv[:, lh0 * 64: lh0 * 64 + 256]),
            "wo": _bf(Wo[lh0 * 64: lh0 * 64 + 256, :]),
            "bqkA": _bf(bqkA),
            "bqkAn": _bf(-bqkA),
            "bqkB": _bf(bqkB),
            "cosr": cosr,
            "sinr": sinr,
            "wm4": wm4,
            "i4q4": i4q4,
            "i4kn2": i4kn2,
            "gsum": gsum,
            "bsum4": bsum4,
            "maskd": maskd,
        })
    return in_maps


def kernel(x, Wq, Wk, Wv, Wo, Wqm, Wkm, Wmetric, temperature, **_):
    from concourse import bass_utils

    temp = float(np.asarray(temperature))
    key = ("nc", temp)
    if key not in _CACHE:
        _CACHE[key] = _build(temp)
        _CACHE["nc"] = _CACHE[key]
    nc = _CACHE[key]

    in_maps = _prep_inputs(x, Wq, Wk, Wv, Wo, Wqm, Wkm, Wmetric, temperature)
    res = bass_utils.run_bass_kernel_spmd(nc, in_maps,
                                          core_ids=list(range(NCORE)))
    y = np.zeros((B, T, DM), np.float32)
    for b in range(B):
        acc = res.results[4 * b]["y"].astype(np.float32)
        for g in range(1, 4):
            acc = acc + res.results[4 * b + g]["y"].astype(np.float32)
        y[b] = acc
    return y
